# revision 6
# baseline (speedup 1.0000x reference)
"""Trainium2 Bass kernel for nn_CrossAttention_46540265619919.

Cross-attention with gene-axis pre-reduction, causal softmax, residual +
LayerNorm.  Full (unsharded) inputs in, full output out; internally sharded
across 8 NeuronCores as (batch b, L-half h): core c -> b = c//2, h = c%2.

v2 design (vs the gene-split baseline):
- context_key is split by K-half (full gene axis per core): each core
  tree-reduces its own half's k_red locally, so the *local* half of every
  scores matmul never waits on communication.  The remote half is recovered
  from a 64 KiB pair AllReduce via remote = sum - local (SPMD-clean).
  Softmax/attention run in a per-core permuted k-order (local half first);
  the host permutes cv and the causal mask to match, which is output-
  invariant since softmax+weighted-sum don't care about key order.
- cv / x / mask / out travel as bf16 (halves their DMA), scores/softmax
  stay f32 (softmax here is saturated; score error must stay << top-2 gap).
- Gene-reduction tree level 1 is split vector(40 genes):gpsimd(24) to keep
  the DVE (the pacing engine) ahead of the DMA stream.
- One 4 MiB dma_start per 128-gene chunk (32 KiB contiguous per partition
  row) to minimize descriptor count.

Self-contained: hardcodes all shapes; no sibling imports.
"""

import os
from contextlib import ExitStack

import numpy as np
import ml_dtypes

import concourse.bass as bass
import concourse.tile as tile
from concourse import bacc, mybir
from concourse.bass_utils import run_bass_kernel_spmd

F32 = mybir.dt.float32
BF16 = mybir.dt.bfloat16
AX = mybir.AxisListType
OP = mybir.AluOpType
AF = mybir.ActivationFunctionType

# Problem shape (fixed).
B, L, K, GT, GC, D = 4, 512, 512, 512, 256, 64
NCORES = 8
LLOC = L // 2          # 256 L-rows per core
LT = LLOC // 128       # 2 l-tiles of 128 rows
KLOC = K // 2          # 256 K-rows reduced locally per core
MASK_PENALTY = 1.0e9
LN_EPS = 1e-3
BF = ml_dtypes.bfloat16

LAST_RESULTS = None    # BassKernelResults of the most recent run (for test harness)
_CACHED_NC = None


def _ensure_trace_hook():
    """If NTFF tracing is requested but this image's `antenv` lacks
    `axon_hooks`, synthesize it from trn_boot's ctypes path so
    run_bass_kernel_spmd's trace branch doesn't crash. Best-effort."""
    try:
        import antenv.axon_hooks  # noqa: F401
        return
    except ImportError:
        pass
    try:
        import sys
        import types
        import trn_agent_boot.trn_boot as tb
        import concourse.bass_utils as bu
        hook = tb._ntff_profile_via_ctypes("/opt/axon/libaxon_pjrt.so")
        mod = types.ModuleType("antenv.axon_hooks")
        mod.get_axon_ntff_profile_hook = lambda: hook
        mod.set_axon_ntff_profile_hook = lambda h: None
        sys.modules["antenv.axon_hooks"] = mod
        bu.upload_artifacts = lambda tmpdir: tmpdir  # no fish creds in-container
    except Exception:
        os.environ["BASS_NEVER_TRACE"] = "1"  # fall back: run untraced


def _build_program():
    """Build + compile the per-core SPMD Tile program."""
    nc = bacc.Bacc(
        "TRN2",
        target_bir_lowering=False,
        debug=False,
        num_devices=NCORES,
    )

    xq_d = nc.dram_tensor("xq", [LLOC, GT, D], F32, kind="ExternalInput").ap()
    ck_d = nc.dram_tensor("ck", [KLOC, GC, D], F32, kind="ExternalInput").ap()
    cv_d = nc.dram_tensor("cv", [K, GT], BF16, kind="ExternalInput").ap()
    x_d = nc.dram_tensor("xres", [LLOC, GT], BF16, kind="ExternalInput").ap()
    mask_d = nc.dram_tensor("mask", [LLOC, K], BF16, kind="ExternalInput").ap()
    out_d = nc.dram_tensor("out", [LLOC, GT], BF16, kind="ExternalOutput").ap()

    with tile.TileContext(nc) as tc, ExitStack() as ctx:
        const = ctx.enter_context(tc.tile_pool(name="const", bufs=1))
        stream = ctx.enter_context(tc.tile_pool(name="stream", bufs=4))
        work = ctx.enter_context(tc.tile_pool(name="work", bufs=2))
        smalls = ctx.enter_context(tc.tile_pool(name="smalls", bufs=2))
        qpool = ctx.enter_context(tc.tile_pool(name="qpool", bufs=4))
        ps_loc = ctx.enter_context(tc.tile_pool(name="ps_loc", bufs=2, space="PSUM"))
        ps_rem = ctx.enter_context(tc.tile_pool(name="ps_rem", bufs=2, space="PSUM"))
        ps_att = ctx.enter_context(tc.tile_pool(name="ps_att", bufs=2, space="PSUM"))
        ps_tp = ctx.enter_context(tc.tile_pool(name="ps_tp", bufs=2, space="PSUM"))
        dram = ctx.enter_context(tc.tile_pool(name="dram", bufs=1, space="DRAM"))

        def tree128(t, out_ap):
            """Sum t[128, 128, D] over the gene axis into out_ap[128, D].

            Level 1 is split vector:gpsimd ~2:1 (their f32-add throughput
            ratio) so the DVE stays ahead of the DMA stream; the rest of the
            halving tree plus the final short strided reduce stay on DVE.
            """
            nc.vector.tensor_add(t[:, 0:40, :], t[:, 0:40, :], t[:, 64:104, :])
            nc.gpsimd.tensor_add(t[:, 40:64, :], t[:, 40:64, :], t[:, 104:128, :])
            n = 64
            while n > 8:
                half = n // 2
                nc.vector.tensor_add(t[:, 0:half, :], t[:, 0:half, :], t[:, half:n, :])
                n = half
            nc.vector.tensor_reduce(
                out_ap, t[:, 0:n, :].rearrange("p g d -> p d g"), axis=AX.X, op=OP.add
            )

        # ---- constants
        ones = const.tile([128, 128], F32, tag="ones")
        ident = const.tile([128, 128], F32, tag="ident")
        eps_b = const.tile([128, 1], F32, tag="eps_b")
        nc.vector.memset(eps_b[:], LN_EPS)
        nc.vector.memset(ones[:], 1.0)
        nc.gpsimd.affine_select(
            ident[:], ones[:],
            pattern=[[-1, 128]], base=0, channel_multiplier=1,
            compare_op=OP.is_equal, fill=0.0,
        )

        # ---- local k_red: tree-reduce the FULL gene axis of this core's
        # K-half.  kloc[:, kc, :] = k_red for local k-chunk kc.
        kloc = const.tile([128, 2, D], F32, tag="kloc")
        for kc in range(2):
            parts = []
            for gh in range(2):
                t = stream.tile([128, 128, D], F32, tag="stream")
                nc.sync.dma_start(
                    t[:], ck_d[kc * 128:(kc + 1) * 128, gh * 128:(gh + 1) * 128, :]
                )
                p = qpool.tile([128, D], F32, tag="part", bufs=4)
                tree128(t, p[:])
                parts.append(p)
            nc.vector.tensor_add(kloc[:, kc, :], parts[0][:], parts[1][:])

        # pair AllReduce of the local k_red halves: sum = local + remote,
        # so remote = sum - local (recovered on-device; SPMD-uniform).
        kred_in = dram.tile([128, 2, D], F32, tag="kred_in")
        kred_out = dram.tile([128, 2, D], F32, tag="kred_out")
        nc.sync.dma_start(kred_in[:], kloc[:])
        nc.gpsimd.collective_compute(
            "AllReduce",
            OP.add,
            replica_groups=[[2 * b, 2 * b + 1] for b in range(B)],
            ins=[kred_in.opt()],
            outs=[kred_out.opt()],
        )
        ksum = const.tile([128, 2, D], F32, tag="ksum")
        nc.sync.dma_start(ksum[:], kred_out[:])
        krem = const.tile([128, 2, D], F32, tag="krem")

        # context_value resident in SBUF as bf16 (host already permuted rows
        # to this core's k-order).
        cv_sb = const.tile([128, 4, GT], BF16, tag="cv")
        for kc in range(4):
            nc.sync.dma_start(cv_sb[:, kc, :], cv_d[kc * 128:(kc + 1) * 128, :])

        # k_redT [d=64, K]: cols 0:256 local halves, 256:512 remote halves.
        k_redT = const.tile([D, K], F32, tag="k_redT")
        for kc in range(2):
            tp = ps_tp.tile([D, 128], F32, tag="tp")
            nc.tensor.transpose(tp[:], kloc[:, kc, :], ident[:])
            nc.scalar.copy(k_redT[:, kc * 128:(kc + 1) * 128], tp[:])

        # ---- per l-tile pipeline ----
        for lt in range(LT):
            lsl = slice(lt * 128, (lt + 1) * 128)

            psl = ps_loc.tile([128, KLOC], F32, tag="psl")
            qTs = []
            for c in range(4):
                t = stream.tile([128, 128, D], F32, tag="stream")
                nc.sync.dma_start(t[:], xq_d[lsl, c * 128:(c + 1) * 128, :])
                qp = qpool.tile([128, D], F32, tag="qp", bufs=6)
                tree128(t, qp[:])
                tq = ps_tp.tile([D, 128], F32, tag="tp")
                nc.tensor.transpose(tq[:], qp[:], ident[:])
                qT = qpool.tile([D, 128], F32, tag="qT", bufs=10)
                nc.scalar.copy(qT[:], tq[:])
                nc.tensor.matmul(
                    psl[:], qT[:], k_redT[:, 0:KLOC], start=(c == 0), stop=(c == 3)
                )
                qTs.append(qT)

            if lt == 0:
                # remote k_red halves.  Emitted here (not earlier) so the
                # gpsimd/PE queues reach these ops only after the lt=0 chunk
                # work, by which time the AllReduce has landed -- the
                # in-order engines never stall the stream on comms.
                nc.gpsimd.tensor_sub(krem[:], ksum[:], kloc[:])
                for kc in range(2):
                    tp = ps_tp.tile([D, 128], F32, tag="tp")
                    nc.tensor.transpose(tp[:], krem[:, kc, :], ident[:])
                    nc.scalar.copy(k_redT[:, KLOC + kc * 128:KLOC + (kc + 1) * 128], tp[:])

            psr = ps_rem.tile([128, KLOC], F32, tag="psr")
            for c in range(4):
                nc.tensor.matmul(
                    psr[:], qTs[c][:], k_redT[:, KLOC:K], start=(c == 0), stop=(c == 3)
                )

            # masked scores in SBUF: s = scores + mask  (mask is 0 / -1e9)
            mask_t = smalls.tile([128, K], BF16, tag="mask")
            nc.sync.dma_start(mask_t[:], mask_d[lsl, :])
            s_sb = work.tile([128, K], F32, tag="s_sb")
            nc.vector.scalar_tensor_tensor(
                s_sb[:, 0:KLOC], psl[:], 1.0, mask_t[:, 0:KLOC],
                op0=OP.mult, op1=OP.add,
            )
            nc.vector.scalar_tensor_tensor(
                s_sb[:, KLOC:K], psr[:], 1.0, mask_t[:, KLOC:K],
                op0=OP.mult, op1=OP.add,
            )

            # softmax pieces: negmax, w = exp(s - max), denom = sum w
            negmax = smalls.tile([128, 1], F32, tag="negmax")
            nc.vector.tensor_reduce(
                negmax[:], s_sb[:], axis=AX.X, op=OP.max, negate=True
            )
            w = work.tile([128, K], F32, tag="w")
            denom = smalls.tile([128, 1], F32, tag="denom")
            nc.scalar.activation(
                w[:], s_sb[:], AF.Exp, bias=negmax[:], scale=1.0, accum_out=denom[:]
            )
            recip = smalls.tile([128, 1], F32, tag="recip")
            nc.vector.reciprocal(recip[:], denom[:])

            # w^T chunks [k=128, l=128] via TensorE transpose, cast to bf16
            wT = work.tile([128, 4, 128], BF16, tag="wT")
            for kc in range(4):
                tw = ps_tp.tile([128, 128], F32, tag="tp")
                nc.tensor.transpose(tw[:], w[:, kc * 128:(kc + 1) * 128], ident[:])
                nc.scalar.copy(wT[:, kc, :], tw[:])

            # attn [128, 512] = w @ cv   (unnormalized; bf16 x bf16 -> f32)
            ps_a = ps_att.tile([128, GT], F32, tag="pa")
            for kc in range(4):
                nc.tensor.matmul(
                    ps_a[:], wT[:, kc, :], cv_sb[:, kc, :],
                    start=(kc == 0), stop=(kc == 3),
                )

            # y = attn * recip + x
            x_t = smalls.tile([128, GT], BF16, tag="x_t")
            nc.sync.dma_start(x_t[:], x_d[lsl, :])
            y = work.tile([128, GT], F32, tag="y")
            nc.vector.scalar_tensor_tensor(
                y[:], ps_a[:], recip[:], x_t[:], op0=OP.mult, op1=OP.add
            )

            # LayerNorm stats via bn_stats/bn_aggr -> [mean, var]
            stats = smalls.tile([128, 6], F32, tag="stats")
            nc.vector.bn_stats(stats[:], y[:])
            mv = smalls.tile([128, 2], F32, tag="mv")
            nc.vector.bn_aggr(mv[:], stats[:])
            # rstd = 1/sqrt(var + eps)
            std = smalls.tile([128, 1], F32, tag="std")
            nc.scalar.activation(std[:], mv[:, 1:2], AF.Sqrt, bias=eps_b[:], scale=1.0)
            rstd = smalls.tile([128, 1], F32, tag="rstd")
            nc.vector.reciprocal(rstd[:], std[:])

            # out = (y - mean) * rstd   (gamma/beta applied host-side)
            o_t = work.tile([128, GT], BF16, tag="o_t")
            nc.vector.tensor_scalar(
                o_t[:], y[:], mv[:, 0:1], rstd[:], op0=OP.subtract, op1=OP.mult
            )
            nc.sync.dma_start(out_d[lsl, :], o_t[:])

    nc.compile()
    return nc


def _get_nc():
    global _CACHED_NC
    if _CACHED_NC is None:
        _CACHED_NC = _build_program()
    return _CACHED_NC


def _perm(h: int) -> np.ndarray:
    """Per-core k-order: local K-half first, remote half second."""
    loc = np.arange(h * KLOC, (h + 1) * KLOC)
    rem = np.arange((1 - h) * KLOC, (2 - h) * KLOC)
    return np.concatenate([loc, rem])


def _causal_mask(h: int) -> np.ndarray:
    """Causal mask in this core's permuted k-order, bf16."""
    lg = h * LLOC + np.arange(LLOC)[:, None]
    kk = _perm(h)[None, :]
    return np.where(kk <= lg, 0.0, -MASK_PENALTY).astype(BF)


_MASKS = {h: _causal_mask(h) for h in range(2)}
_PERMS = {h: _perm(h) for h in range(2)}


def kernel(x, x_query, context_key, context_value, gamma, beta):
    global LAST_RESULTS
    x = np.asarray(x, np.float32)
    x_query = np.asarray(x_query, np.float32)
    context_key = np.asarray(context_key, np.float32)
    context_value = np.asarray(context_value, np.float32)
    gamma = np.asarray(gamma, np.float32)
    beta = np.asarray(beta, np.float32)

    nc = _get_nc()
    in_maps = []
    for c in range(NCORES):
        b, h = c // 2, c % 2
        sl = slice(h * LLOC, (h + 1) * LLOC)
        ksl = slice(h * KLOC, (h + 1) * KLOC)
        in_maps.append({
            "xq": np.ascontiguousarray(x_query[b, sl]),
            "ck": np.ascontiguousarray(context_key[b, ksl]),
            "cv": np.ascontiguousarray(context_value[b][_PERMS[h]]).astype(BF),
            "xres": np.ascontiguousarray(x[b, sl]).astype(BF),
            "mask": _MASKS[h],
        })

    if os.environ.get("KERNEL_TRACE") or os.environ.get("BASS_TRACE"):
        _ensure_trace_hook()
    res = run_bass_kernel_spmd(
        nc,
        in_maps,
        core_ids=list(range(NCORES)),
        trace=bool(os.environ.get("KERNEL_TRACE")),
    )
    LAST_RESULTS = res

    out = np.empty((B, L, GT), np.float32)
    for c, r in enumerate(res.results):
        b, h = c // 2, c % 2
        out[b, h * LLOC:(h + 1) * LLOC] = np.asarray(r["out"]).astype(np.float32)
    # LN affine (gamma/beta broadcast over the last axis) applied on host.
    out = out * gamma + beta
    return out.astype(np.float32)
